# revision 27
# baseline (speedup 1.0000x reference)
"""Non-local block (nn_Non_localBlock) for TRN2, data-parallel over batch on 8 cores.

Reference math (per batch item, channel-major, x in [C=128, N=4096]):
    theta = w_theta @ x + b_theta                  [64, N]
    phi   = maxpool2(w_phi @ x + b_phi)            [64, N/4]
    g     = maxpool2(w_g   @ x + b_g)              [64, N/4]
    scores = theta^T @ phi                         [N, N/4]
    out   = scores @ g^T                           [N, 64]     (softmax bug preserved:
    y     = x + w_out @ out^T + b_out              [C, N]       raw scores are used)

Because the raw scores feed the second matmul, associativity collapses the
attention:  (theta^T @ phi) @ g^T = theta^T @ (phi @ g^T) = theta^T @ M with
M = phi @ g^T in [64, 64].  Folding the 1x1 convs gives a per-batch fused map
    y = (I + W_eff) @ x + b_eff
    W_eff = w_out @ M^T @ w_theta   [128, 128]
    b_eff = w_out @ M^T @ b_theta + b_out
so the device kernel only computes the phi/g convs + pools (to get M), three
tiny matmuls for W_eff/b_eff, and one fused [128,128] @ [128,4096] matmul.

x streams in as bf16 (halves the input DMA) and the conv/final matmuls run
in bf16; the residual rides inside W' = I + W_eff so the x-passthrough error
is just bf16 rounding of x (~1e-6 of output scale). The small W_eff ladder
stays fp32r. Measured end-to-end scale-relative absmax error ~1e-3.
"""

import numpy as np
from contextlib import ExitStack

B, C, H, W = 16, 128, 64, 64
CH = C // 2            # 64 bottleneck channels
N = H * W              # 4096 spatial positions
NCORES = 8
BPC = B // NCORES      # batches per core
NCHUNK = 512           # free-dim chunk for the big matmuls (one PSUM bank)
NCH = N // NCHUNK      # 8 chunks
XCH = 1024             # x/y DMA chunk (1 MB per load)
NX = N // XCH          # 4 DMA chunks
PO = N // 4            # 1024 pooled positions

_prog_cache = {}


def _build_program():
    """Build + compile the per-core Bass/Tile program (shared by all 8 cores)."""
    import concourse.bass as bass
    from concourse import bacc, mybir, tile
    from concourse.masks import make_identity

    f32 = mybir.dt.float32
    f32r = mybir.dt.float32r
    bf16 = mybir.dt.bfloat16

    nc = bacc.Bacc("TRN2", target_bir_lowering=False, debug=False)

    x_d = nc.dram_tensor("x", [BPC, C, N], bf16, kind="ExternalInput").ap()
    wpg_d = nc.dram_tensor("w_pg", [C, C], bf16, kind="ExternalInput").ap()
    bpg_d = nc.dram_tensor("b_pg", [C, 1], f32, kind="ExternalInput").ap()
    wth_d = nc.dram_tensor("w_th", [CH, C], f32r, kind="ExternalInput").ap()
    bth_d = nc.dram_tensor("b_th", [CH, 2], f32r, kind="ExternalInput").ap()
    wot_d = nc.dram_tensor("w_ot", [CH, C], f32r, kind="ExternalInput").ap()
    bot_d = nc.dram_tensor("b_ot", [C, 1], f32, kind="ExternalInput").ap()
    y_d = nc.dram_tensor("y", [BPC, C, N], f32, kind="ExternalOutput").ap()

    with tile.TileContext(nc) as tc, ExitStack() as ctx:
        consts = ctx.enter_context(tc.tile_pool(name="consts", bufs=1))
        xpool = ctx.enter_context(tc.tile_pool(name="xp", bufs=NCH))
        ypool = ctx.enter_context(tc.tile_pool(name="yp", bufs=2 * NX))
        mids = ctx.enter_context(tc.tile_pool(name="mids", bufs=2))
        tmps = ctx.enter_context(tc.tile_pool(name="tmps", bufs=4))
        smalls = ctx.enter_context(tc.tile_pool(name="smalls", bufs=2))
        ps_big = ctx.enter_context(tc.tile_pool(name="ps_big", bufs=4, space="PSUM"))
        ps_tp = ctx.enter_context(tc.tile_pool(name="ps_tp", bufs=2, space="PSUM"))
        ps_sm = ctx.enter_context(tc.tile_pool(name="ps_sm", bufs=2, space="PSUM"))

        # weights go through the ACT HWDGE queue so the Sync queue can start
        # streaming x immediately; identity setup runs on GpSimd.
        wpg = consts.tile([C, C], bf16)
        nc.scalar.dma_start(wpg[:], wpg_d)
        bpg = consts.tile([C, 1], f32)
        nc.scalar.dma_start(bpg[:], bpg_d)
        wth = consts.tile([CH, C], f32r)
        nc.scalar.dma_start(wth[:], wth_d)
        bth = consts.tile([CH, 2], f32r)
        nc.scalar.dma_start(bth[:], bth_d)
        wot = consts.tile([CH, C], f32r)
        nc.scalar.dma_start(wot[:], wot_d)
        bot = consts.tile([C, 1], f32)
        nc.scalar.dma_start(bot[:], bot_d)
        ident = consts.tile([128, 128], f32)
        make_identity(nc, ident[:])
        identb = consts.tile([128, 128], bf16)
        make_identity(nc, identb[:])

        # x loads right behind the (tiny) weight loads. Batch 0 streams in
        # 512 KB chunks (the first conv only waits ~1.4 us), batch 1 in 1 MB
        # chunks (arrives during batch 0 compute anyway).
        x_chunk_cols = {0: NCHUNK, 1: XCH}
        x_sb = []
        for b in range(BPC):
            cols = x_chunk_cols[b]
            chunks = []
            for j in range(N // cols):
                t = xpool.tile([C, cols], bf16, tag=f"x{b}", name=f"x_{b}_{j}")
                nc.sync.dma_start(t[:], x_d[b, :, j * cols:(j + 1) * cols])
                chunks.append(t)
            x_sb.append(chunks)

        def xs(b, i):
            """x slice for conv chunk i of batch b: [128, NCHUNK]."""
            per = x_chunk_cols[b] // NCHUNK
            return x_sb[b][i // per][:, (i % per) * NCHUNK:(i % per + 1) * NCHUNK]

        pooleds = {}

        def emit_conv(b, braid=None):
            # phi/g conv (both halves in one matmul: out partitions 0:64 phi,
            # 64:128 g) + 2x2/2 maxpool over the [H, W] spatial grid.
            # Pool order: h-pairs first (ACT stages even rows with a
            # contiguous inner-64 read; DVE maxes in the odd rows), then
            # w-pairs on DVE (SBUF only).
            pooled_f = mids.tile([C, PO], f32, tag="pooled_f")
            for i in range(NCH):
                cps = ps_big.tile([C, NCHUNK], f32, tag="big")
                nc.tensor.matmul(
                    cps[:], lhsT=wpg[:], rhs=xs(b, i), start=True, stop=True,
                )
                v = cps[:].rearrange("p (h w) -> p h w", w=W)       # [128, 8, 64]
                u = tmps.tile([C, 4, W], f32, tag="u")              # [128, 4, 64]
                nc.scalar.copy(u[:], v[:, ::2, :])
                nc.vector.tensor_max(u[:], u[:], v[:, 1::2, :])
                po = pooled_f[:, i * 128:(i + 1) * 128].rearrange(
                    "p (h w) -> p h w", w=W // 2
                )                                                   # [128, 4, 32]
                nc.vector.tensor_max(po, u[:, :, ::2], u[:, :, 1::2])
                if braid is not None:
                    braid(i)
            # channel bias commutes with the spatial max; fold the bias add
            # with the bf16 rounding for the M-path.
            pooled = mids.tile([C, PO], bf16, tag="pooled", name=f"pooled_{b}")
            nc.vector.tensor_scalar_add(pooled[:], pooled_f[:], bpg[:])
            pooleds[b] = pooled

        def emit_tp_m(b):
            """PE-transpose pooled[b] chunkwise (bf16: 1 cyc/row; four land in
            one PSUM bank -> one ACT copy per four) into tsb (chunk j is
            [spatial 128, phi 0:64 | g 64:128]), then accumulate
            M^T = g_pool @ phi_pool^T over the spatial chunks."""
            pooled = pooleds[b]
            tsb = mids.tile([C, PO], bf16, tag="tsb", name=f"tsb_{b}")
            for k in range(NCH // 4):
                tp = ps_tp.tile([128, 4, 128], bf16, tag="tp", name=f"tp_{b}_{k}")
                for j in range(4):
                    i = k * 4 + j
                    nc.tensor.transpose(
                        tp[:, j, :], pooled[:, i * 128:(i + 1) * 128], identb[:]
                    )
                nc.scalar.copy(
                    tsb[:, k * 512:(k + 1) * 512],
                    tp[:].rearrange("p a b -> p (a b)"),
                )
            mt_ps = ps_sm.tile([CH, CH], f32, tag="sm", name=f"mt_ps_{b}")
            for i in range(NCH):
                nc.tensor.matmul(
                    mt_ps[:],
                    lhsT=tsb[:, i * 128 + CH:(i + 1) * 128],
                    rhs=tsb[:, i * 128:i * 128 + CH],
                    start=(i == 0),
                    stop=(i == NCH - 1),
                )
            mt = smalls.tile([CH, CH], f32r, tag="mt", name=f"mt_{b}")
            nc.vector.tensor_copy(mt[:], mt_ps[:])
            return mt

        def emit_ladder(b, mt):
            """A^T = M @ w_out^T, then W_eff^T (+I for the residual) and
            b_eff = A @ b_theta + b_out."""
            at_ps = ps_sm.tile([CH, C], f32, tag="sm", name=f"at_ps_{b}")
            nc.tensor.matmul(
                at_ps[:], lhsT=mt[:], rhs=wot[:], start=True, stop=True,
            )
            at = smalls.tile([CH, C], f32r, tag="at", name=f"at_{b}")
            nc.vector.tensor_copy(at[:], at_ps[:])

            w_ps = ps_sm.tile([C, C], f32, tag="sm", name=f"w_ps_{b}")
            nc.tensor.matmul(
                w_ps[:], lhsT=wth[:], rhs=at[:], start=True, stop=True,
            )
            weff = smalls.tile([C, C], bf16, tag="weff", name=f"weff_{b}")
            nc.vector.tensor_add(weff[:], w_ps[:], ident[:])

            # fp32r matmul needs an even moving free-count: b_theta is sent
            # duplicated as [64, 2]; use column 0 of the result.
            be_ps = ps_sm.tile([C, 2], f32, tag="sm", name=f"be_ps_{b}")
            nc.tensor.matmul(
                be_ps[:], lhsT=at[:], rhs=bth[:], start=True, stop=True,
            )
            beff = smalls.tile([C, 1], f32, tag="beff", name=f"beff_{b}")
            nc.vector.tensor_add(beff[:], be_ps[:, 0:1], bot[:])
            return weff, beff

        def emit_final_chunk(b, weff, beff, i):
            """One chunk of y = W'^T.T @ x + b_eff (residual inside W').
            Epilogue alternates ACT/DVE; y streams out in 512 KB chunks on
            the Sync queue so the writeback overlaps the remaining compute."""
            yps = ps_big.tile([C, NCHUNK], f32, tag="big", name=f"yps_{b}_{i}")
            nc.tensor.matmul(
                yps[:], lhsT=weff[:], rhs=xs(b, i), start=True, stop=True,
            )
            y_sb = ypool.tile([C, NCHUNK], f32, tag="y", name=f"y_{b}_{i}")
            if i % 2 == 0:
                nc.scalar.add(y_sb[:], yps[:], beff[:])
            else:
                nc.vector.tensor_scalar_add(y_sb[:], yps[:], beff[:])
            nc.sync.dma_start(
                y_d[b, :, i * NCHUNK:(i + 1) * NCHUNK], y_sb[:]
            )

        def emit_final(b, weff, beff):
            for i in range(NCH):
                emit_final_chunk(b, weff, beff, i)

        # PE stream: conv0 | tp0 M0 ladder0 (fills the x-DMA-paced gaps) |
        # conv1 | tp1 M1 (so batch 1's M is ready early) | final0 (ladder1's
        # copies overlap it) | ladder1 | final1.
        emit_conv(0)
        mt0 = emit_tp_m(0)
        wb0 = emit_ladder(0, mt0)
        emit_conv(1)
        mt1 = emit_tp_m(1)
        emit_final(0, *wb0)
        wb1 = emit_ladder(1, mt1)
        emit_final(1, *wb1)

    nc.compile()
    return nc


def _get_program():
    if "nc" not in _prog_cache:
        _prog_cache["nc"] = _build_program()
    return _prog_cache["nc"]


def _make_in_maps(inputs):
    import ml_dtypes

    f = lambda a: np.ascontiguousarray(np.asarray(a, dtype=np.float32))
    bf = lambda a: np.ascontiguousarray(np.asarray(a).astype(ml_dtypes.bfloat16))
    x = bf(np.asarray(inputs["x"], dtype=np.float32)).reshape(NCORES, BPC, C, N)
    w_pg = bf(np.concatenate([np.asarray(inputs["w_phi"]), np.asarray(inputs["w_g"])], axis=0).T)
    b_pg = f(np.concatenate([np.asarray(inputs["b_phi"]), np.asarray(inputs["b_g"])])[:, None])
    w_th = f(inputs["w_theta"])
    b_th = f(np.repeat(np.asarray(inputs["b_theta"])[:, None], 2, axis=1))
    w_ot = f(np.asarray(inputs["w_out"]).T)
    b_ot = f(np.asarray(inputs["b_out"])[:, None])
    return [
        {
            "x": np.ascontiguousarray(x[i]),
            "w_pg": w_pg,
            "b_pg": b_pg,
            "w_th": w_th,
            "b_th": b_th,
            "w_ot": w_ot,
            "b_ot": b_ot,
        }
        for i in range(NCORES)
    ]


def _run(inputs, trace=False, **kwargs):
    from concourse.bass_utils import run_bass_kernel_spmd

    nc = _get_program()
    in_maps = _make_in_maps(inputs)
    res = run_bass_kernel_spmd(
        nc, in_maps, core_ids=list(range(NCORES)), trace=trace, **kwargs
    )
    y = np.stack([r["y"] for r in res.results], axis=0)
    return y.reshape(B, C, H, W), res


def _run_subprocess(inputs):
    """Fallback: rerun in a fresh process (fresh PJRT client) with core reset,
    for the rare NRT_EXEC_UNIT_UNRECOVERABLE device wedge."""
    import os
    import subprocess
    import sys
    import tempfile

    with tempfile.TemporaryDirectory() as td:
        inp = os.path.join(td, "in.npz")
        outp = os.path.join(td, "out.npy")
        np.savez(inp, **{k: np.asarray(v) for k, v in inputs.items()})
        code = (
            "import numpy as np, sys\n"
            f"sys.path.insert(0, {os.path.dirname(os.path.abspath(__file__))!r})\n"
            "import kernel as K\n"
            f"d = dict(np.load({inp!r}))\n"
            "y, _ = K._run(d, trace=False)\n"
            f"np.save({outp!r}, y)\n"
        )
        env = dict(os.environ)
        env["NEURON_RT_RESET_CORES"] = "1"
        env["BASS_KERNEL_NO_SUBPROCESS_RETRY"] = "1"
        subprocess.run(
            [sys.executable, "-c", code], env=env, check=True, timeout=1200
        )
        return np.load(outp)


def kernel(**inputs) -> np.ndarray:
    import os

    try:
        y, _ = _run(inputs, trace=False)
        return y
    except Exception:
        if os.environ.get("BASS_KERNEL_NO_SUBPROCESS_RETRY"):
            raise
        return _run_subprocess(inputs)


# revision 29
# speedup vs baseline: 1.0511x; 1.0511x over previous
"""Non-local block (nn_Non_localBlock) for TRN2, data-parallel over batch on 8 cores.

Reference math (per batch item, channel-major, x in [C=128, N=4096]):
    theta = w_theta @ x + b_theta                  [64, N]
    phi   = maxpool2(w_phi @ x + b_phi)            [64, N/4]
    g     = maxpool2(w_g   @ x + b_g)              [64, N/4]
    scores = theta^T @ phi                         [N, N/4]
    out   = scores @ g^T                           [N, 64]     (softmax bug preserved:
    y     = x + w_out @ out^T + b_out              [C, N]       raw scores are used)

Because the raw scores feed the second matmul, associativity collapses the
attention:  (theta^T @ phi) @ g^T = theta^T @ (phi @ g^T) = theta^T @ M with
M = phi @ g^T in [64, 64].  Folding the 1x1 convs gives a per-batch fused map
    y = (I + W_eff) @ x + b_eff
    W_eff = w_out @ M^T @ w_theta   [128, 128]
    b_eff = w_out @ M^T @ b_theta + b_out
so the device kernel only computes the phi/g convs + pools (to get M), three
tiny matmuls for W_eff/b_eff, and one fused [128,128] @ [128,4096] matmul.

x streams in as bf16 (halves the input DMA) and the conv/final matmuls run
in bf16; the residual rides inside W' = I + W_eff so the x-passthrough error
is just bf16 rounding of x (~1e-6 of output scale). The small W_eff ladder
stays fp32r. Measured end-to-end scale-relative absmax error ~1e-3.
"""

import numpy as np
from contextlib import ExitStack

B, C, H, W = 16, 128, 64, 64
CH = C // 2            # 64 bottleneck channels
N = H * W              # 4096 spatial positions
NCORES = 8
BPC = B // NCORES      # batches per core
NCHUNK = 512           # free-dim chunk for the big matmuls (one PSUM bank)
NCH = N // NCHUNK      # 8 chunks
XCH = 1024             # x/y DMA chunk (1 MB per load)
NX = N // XCH          # 4 DMA chunks
PO = N // 4            # 1024 pooled positions

_prog_cache = {}


def _build_program():
    """Build + compile the per-core Bass/Tile program (shared by all 8 cores)."""
    import concourse.bass as bass
    from concourse import bacc, mybir, tile
    from concourse.masks import make_identity

    f32 = mybir.dt.float32
    f32r = mybir.dt.float32r
    bf16 = mybir.dt.bfloat16

    nc = bacc.Bacc("TRN2", target_bir_lowering=False, debug=False)

    x_d = nc.dram_tensor("x", [BPC, C, N], bf16, kind="ExternalInput").ap()
    wpg_d = nc.dram_tensor("w_pg", [C, C], bf16, kind="ExternalInput").ap()
    bpg_d = nc.dram_tensor("b_pg", [C, 1], f32, kind="ExternalInput").ap()
    wth_d = nc.dram_tensor("w_th", [CH, C], f32r, kind="ExternalInput").ap()
    bth_d = nc.dram_tensor("b_th", [CH, 2], f32r, kind="ExternalInput").ap()
    wot_d = nc.dram_tensor("w_ot", [CH, C], f32r, kind="ExternalInput").ap()
    bot_d = nc.dram_tensor("b_ot", [C, 1], f32, kind="ExternalInput").ap()
    y_d = nc.dram_tensor("y", [BPC, C, N], f32, kind="ExternalOutput").ap()

    with tile.TileContext(nc) as tc, ExitStack() as ctx:
        consts = ctx.enter_context(tc.tile_pool(name="consts", bufs=1))
        xpool = ctx.enter_context(tc.tile_pool(name="xp", bufs=4))
        ypool = ctx.enter_context(tc.tile_pool(name="yp", bufs=2 * NX))
        mids = ctx.enter_context(tc.tile_pool(name="mids", bufs=2))
        tmps = ctx.enter_context(tc.tile_pool(name="tmps", bufs=4))
        smalls = ctx.enter_context(tc.tile_pool(name="smalls", bufs=2))
        ps_big = ctx.enter_context(tc.tile_pool(name="ps_big", bufs=5, space="PSUM"))
        ps_tp = ctx.enter_context(tc.tile_pool(name="ps_tp", bufs=1, space="PSUM"))
        ps_sm = ctx.enter_context(tc.tile_pool(name="ps_sm", bufs=2, space="PSUM"))

        # weights go through the ACT HWDGE queue so the Sync queue can start
        # streaming x immediately; identity setup runs on GpSimd.
        wpg = consts.tile([C, C], bf16)
        nc.scalar.dma_start(wpg[:], wpg_d)
        bpg = consts.tile([C, 1], f32)
        nc.scalar.dma_start(bpg[:], bpg_d)
        wth = consts.tile([CH, C], f32r)
        nc.scalar.dma_start(wth[:], wth_d)
        bth = consts.tile([CH, 2], f32r)
        nc.scalar.dma_start(bth[:], bth_d)
        wot = consts.tile([CH, C], f32r)
        nc.scalar.dma_start(wot[:], wot_d)
        bot = consts.tile([C, 1], f32)
        nc.scalar.dma_start(bot[:], bot_d)
        ident = consts.tile([128, 128], f32)
        make_identity(nc, ident[:])
        identb = consts.tile([128, 128], bf16)
        make_identity(nc, identb[:])

        # x loads right behind the (tiny) weight loads. bf16 x: batch 0
        # streams in 256 KB chunks (first conv waits ~0.7 us), batch 1 in
        # 512 KB chunks (arrives during batch 0 compute anyway).
        x_chunk_cols = {0: 2 * NCHUNK, 1: 2 * XCH}
        x_sb = []
        for b in range(BPC):
            cols = x_chunk_cols[b]
            chunks = []
            for j in range(N // cols):
                t = xpool.tile([C, cols], bf16, tag=f"x{b}", name=f"x_{b}_{j}")
                nc.sync.dma_start(t[:], x_d[b, :, j * cols:(j + 1) * cols])
                chunks.append(t)
            x_sb.append(chunks)

        def xs(b, i):
            """x slice for conv chunk i of batch b: [128, NCHUNK]."""
            per = x_chunk_cols[b] // NCHUNK
            return x_sb[b][i // per][:, (i % per) * NCHUNK:(i % per + 1) * NCHUNK]

        pooleds = {}

        def emit_conv(b, braid=None):
            # phi/g conv (both halves in one matmul: out partitions 0:64 phi,
            # 64:128 g) + 2x2/2 maxpool over the [H, W] spatial grid.
            # Pool order: h-pairs first (ACT stages even rows with a
            # contiguous inner-64 read; DVE maxes in the odd rows), then
            # w-pairs on DVE (SBUF only).
            pooled_f = mids.tile([C, PO], f32, tag="pooled_f")
            for i in range(NCH):
                cps = ps_big.tile([C, NCHUNK], f32, tag="big")
                nc.tensor.matmul(
                    cps[:], lhsT=wpg[:], rhs=xs(b, i), start=True, stop=True,
                )
                v = cps[:].rearrange("p (h w) -> p h w", w=W)       # [128, 8, 64]
                u = tmps.tile([C, 4, W], f32, tag="u")              # [128, 4, 64]
                nc.scalar.copy(u[:], v[:, ::2, :])
                nc.vector.tensor_max(u[:], u[:], v[:, 1::2, :])
                po = pooled_f[:, i * 128:(i + 1) * 128].rearrange(
                    "p (h w) -> p h w", w=W // 2
                )                                                   # [128, 4, 32]
                nc.vector.tensor_max(po, u[:, :, ::2], u[:, :, 1::2])
                if braid is not None:
                    braid(i)
            # channel bias commutes with the spatial max; fold the bias add
            # with the bf16 rounding for the M-path.
            pooled = mids.tile([C, PO], bf16, tag="pooled", name=f"pooled_{b}")
            nc.vector.tensor_scalar_add(pooled[:], pooled_f[:], bpg[:])
            pooleds[b] = pooled

        def emit_tp_m(b):
            """PE-transpose pooled[b] chunkwise (bf16: 1 cyc/row; four land in
            one PSUM bank -> one ACT copy per four) into tsb (chunk j is
            [spatial 128, phi 0:64 | g 64:128]), then accumulate
            M^T = g_pool @ phi_pool^T over the spatial chunks."""
            pooled = pooleds[b]
            tsb = mids.tile([C, PO], bf16, tag="tsb", name=f"tsb_{b}")
            for k in range(NCH // 4):
                tp = ps_tp.tile([128, 4, 128], bf16, tag="tp", name=f"tp_{b}_{k}")
                for j in range(4):
                    i = k * 4 + j
                    nc.tensor.transpose(
                        tp[:, j, :], pooled[:, i * 128:(i + 1) * 128], identb[:]
                    )
                nc.scalar.copy(
                    tsb[:, k * 512:(k + 1) * 512],
                    tp[:].rearrange("p a b -> p (a b)"),
                )
            mt_ps = ps_sm.tile([CH, CH], f32, tag="sm", name=f"mt_ps_{b}")
            for i in range(NCH):
                nc.tensor.matmul(
                    mt_ps[:],
                    lhsT=tsb[:, i * 128 + CH:(i + 1) * 128],
                    rhs=tsb[:, i * 128:i * 128 + CH],
                    start=(i == 0),
                    stop=(i == NCH - 1),
                )
            mt = smalls.tile([CH, CH], f32r, tag="mt", name=f"mt_{b}")
            nc.vector.tensor_copy(mt[:], mt_ps[:])
            return mt

        def emit_ladder(b, mt):
            """A^T = M @ w_out^T, then W_eff^T (+I for the residual) and
            b_eff = A @ b_theta + b_out."""
            at_ps = ps_sm.tile([CH, C], f32, tag="sm", name=f"at_ps_{b}")
            nc.tensor.matmul(
                at_ps[:], lhsT=mt[:], rhs=wot[:], start=True, stop=True,
            )
            at = smalls.tile([CH, C], f32r, tag="at", name=f"at_{b}")
            nc.vector.tensor_copy(at[:], at_ps[:])

            w_ps = ps_sm.tile([C, C], f32, tag="sm", name=f"w_ps_{b}")
            nc.tensor.matmul(
                w_ps[:], lhsT=wth[:], rhs=at[:], start=True, stop=True,
            )
            weff = smalls.tile([C, C], bf16, tag="weff", name=f"weff_{b}")
            nc.vector.tensor_add(weff[:], w_ps[:], ident[:])

            # fp32r matmul needs an even moving free-count: b_theta is sent
            # duplicated as [64, 2]; use column 0 of the result.
            be_ps = ps_sm.tile([C, 2], f32, tag="sm", name=f"be_ps_{b}")
            nc.tensor.matmul(
                be_ps[:], lhsT=at[:], rhs=bth[:], start=True, stop=True,
            )
            beff = smalls.tile([C, 1], f32, tag="beff", name=f"beff_{b}")
            nc.vector.tensor_add(beff[:], be_ps[:, 0:1], bot[:])
            return weff, beff

        def emit_final_chunk(b, weff, beff, i):
            """One chunk of y = W'^T.T @ x + b_eff (residual inside W').
            Epilogue alternates ACT/DVE; y streams out in 512 KB chunks on
            the Sync queue so the writeback overlaps the remaining compute."""
            yps = ps_big.tile([C, NCHUNK], f32, tag="big", name=f"yps_{b}_{i}")
            nc.tensor.matmul(
                yps[:], lhsT=weff[:], rhs=xs(b, i), start=True, stop=True,
            )
            y_sb = ypool.tile([C, NCHUNK], f32, tag="y", name=f"y_{b}_{i}")
            if i % 4 != 3:
                nc.scalar.add(y_sb[:], yps[:], beff[:])
            else:
                nc.vector.tensor_scalar_add(y_sb[:], yps[:], beff[:])
            nc.sync.dma_start(
                y_d[b, :, i * NCHUNK:(i + 1) * NCHUNK], y_sb[:]
            )

        def emit_final(b, weff, beff):
            for i in range(NCH):
                emit_final_chunk(b, weff, beff, i)

        # PE stream: conv0 | tp0 M0 ladder0 (fills the x-DMA-paced gaps) |
        # conv1 | tp1 M1 (so batch 1's M is ready early) | final0 (ladder1's
        # copies overlap it) | ladder1 | final1.
        emit_conv(0)
        mt0 = emit_tp_m(0)
        wb0 = emit_ladder(0, mt0)
        emit_conv(1)
        mt1 = emit_tp_m(1)
        emit_final(0, *wb0)
        wb1 = emit_ladder(1, mt1)
        emit_final(1, *wb1)

    nc.compile()
    return nc


def _get_program():
    if "nc" not in _prog_cache:
        _prog_cache["nc"] = _build_program()
    return _prog_cache["nc"]


def _make_in_maps(inputs):
    import ml_dtypes

    f = lambda a: np.ascontiguousarray(np.asarray(a, dtype=np.float32))
    bf = lambda a: np.ascontiguousarray(np.asarray(a).astype(ml_dtypes.bfloat16))
    x = bf(np.asarray(inputs["x"], dtype=np.float32)).reshape(NCORES, BPC, C, N)
    w_pg = bf(np.concatenate([np.asarray(inputs["w_phi"]), np.asarray(inputs["w_g"])], axis=0).T)
    b_pg = f(np.concatenate([np.asarray(inputs["b_phi"]), np.asarray(inputs["b_g"])])[:, None])
    w_th = f(inputs["w_theta"])
    b_th = f(np.repeat(np.asarray(inputs["b_theta"])[:, None], 2, axis=1))
    w_ot = f(np.asarray(inputs["w_out"]).T)
    b_ot = f(np.asarray(inputs["b_out"])[:, None])
    return [
        {
            "x": np.ascontiguousarray(x[i]),
            "w_pg": w_pg,
            "b_pg": b_pg,
            "w_th": w_th,
            "b_th": b_th,
            "w_ot": w_ot,
            "b_ot": b_ot,
        }
        for i in range(NCORES)
    ]


def _run(inputs, trace=False, **kwargs):
    from concourse.bass_utils import run_bass_kernel_spmd

    nc = _get_program()
    in_maps = _make_in_maps(inputs)
    res = run_bass_kernel_spmd(
        nc, in_maps, core_ids=list(range(NCORES)), trace=trace, **kwargs
    )
    y = np.stack([r["y"] for r in res.results], axis=0)
    return y.reshape(B, C, H, W), res


def _run_subprocess(inputs):
    """Fallback: rerun in a fresh process (fresh PJRT client) with core reset,
    for the rare NRT_EXEC_UNIT_UNRECOVERABLE device wedge."""
    import os
    import subprocess
    import sys
    import tempfile

    with tempfile.TemporaryDirectory() as td:
        inp = os.path.join(td, "in.npz")
        outp = os.path.join(td, "out.npy")
        np.savez(inp, **{k: np.asarray(v) for k, v in inputs.items()})
        code = (
            "import numpy as np, sys\n"
            f"sys.path.insert(0, {os.path.dirname(os.path.abspath(__file__))!r})\n"
            "import kernel as K\n"
            f"d = dict(np.load({inp!r}))\n"
            "y, _ = K._run(d, trace=False)\n"
            f"np.save({outp!r}, y)\n"
        )
        env = dict(os.environ)
        env["NEURON_RT_RESET_CORES"] = "1"
        env["BASS_KERNEL_NO_SUBPROCESS_RETRY"] = "1"
        subprocess.run(
            [sys.executable, "-c", code], env=env, check=True, timeout=1200
        )
        return np.load(outp)


def kernel(**inputs) -> np.ndarray:
    import os

    try:
        y, _ = _run(inputs, trace=False)
        return y
    except Exception:
        if os.environ.get("BASS_KERNEL_NO_SUBPROCESS_RETRY"):
            raise
        return _run_subprocess(inputs)
